# revision 1
# baseline (speedup 1.0000x reference)
"""Bahdanau (additive) attention kernel for Trainium2, 8 NeuronCores.

Problem shapes: inp (B=4, T=128, D=512), context (B=4, S=512, D=512).
  wq   = inp @ Wq.T + bq                      (B,T,D)
  uh   = context @ Wc.T                       (B,S,D)
  align= einsum('btsd,d->bts', tanh(wq[:,:,None,:]+uh[:,None,:,:]), v)
  a    = softmax(align, -1)                   (B,T,S)
  c    = einsum('bts,bsd->btd', a, context)
  attn = concat([c, inp], -1) @ Wout.T + bout (B,T,D)
Returns (attn, a).

Sharding: 8 cores, core c handles batch b=c//2 and target-half th=c%2
(64 target positions per core). Weights replicated. All layout
transposes are done on the host (numpy), and matrix operands are
pre-cast to fp16 on the host, so the device only streams.

Per-core schedule (ACT-bound; tanh of 16.8M elements is ~110us at 128
lanes x 1.2GHz):
  - uh^T, wq^T via fp16 matmuls (f32 PSUM accumulate)
  - main loop over 16 blocks of 4 target positions:
      DVE tensor_scalar adds broadcast wq[t,:] (f32 scalar) onto fp16
      uh^T at 4x mode; two ACT tanh instructions per block (FD=4096
      each -> fp16) so the PE gets work every ~3.6us and its HAM clock
      stays warm; PE matmuls reduce against v via a shifted-window
      one-hot lhsT (Z[:, 63-t:127-t] has v in column t), accumulating
      align rows into a single [64,512] PSUM tile.
  - batched softmax: DVE reduce_max(negate) -> ACT exp(bias)+accum_out
    -> DVE reciprocal + tensor_scalar_mul
  - PE transposes align -> alignT (fp16), fp16 matmuls for c and the
    output projection (bias via a rank-1 f32 ones x bout matmul).
"""

import numpy as np

import concourse.bacc as bacc
import concourse.tile as tile
from concourse import mybir
from concourse.bass import ds, ts
from concourse.bass_utils import run_bass_kernel_spmd
from concourse.masks import make_identity

F32 = mybir.dt.float32
F16 = mybir.dt.float16

B, T, S, D = 4, 128, 512, 512
N_CORES = 8
TH = T // 2  # 64 target positions per core
NCH = D // 128  # 4 partition chunks of the model dim
TBLK = 4  # target positions per main-loop block
NBLK = TH // TBLK

_NC_CACHE = {}


def _build_nc():
    nc = bacc.Bacc("TRN2", target_bir_lowering=False, debug=False, num_devices=N_CORES)

    inpT = nc.dram_tensor("inpT", [D, TH], F16, kind="ExternalInput")
    ctxT = nc.dram_tensor("ctxT", [D, S], F16, kind="ExternalInput")
    wqT = nc.dram_tensor("wqT", [D, D], F16, kind="ExternalInput")
    wcT = nc.dram_tensor("wcT", [D, D], F16, kind="ExternalInput")
    woutT = nc.dram_tensor("woutT", [2 * D, D], F16, kind="ExternalInput")
    bq = nc.dram_tensor("bq", [D], F32, kind="ExternalInput")
    v = nc.dram_tensor("v", [D], F32, kind="ExternalInput")
    bout = nc.dram_tensor("bout", [D], F32, kind="ExternalInput")
    attn = nc.dram_tensor("attn", [TH, D], F32, kind="ExternalOutput")
    align = nc.dram_tensor("align", [TH, S], F32, kind="ExternalOutput")

    with tile.TileContext(nc) as tc:
        _emit(nc, tc, inpT, ctxT, wqT, wcT, woutT, bq, v, bout, attn, align)
    nc.compile()
    return nc


def _emit(nc, tc, inpT, ctxT, wqT, wcT, woutT, bq, v, bout, attn, align):
    Tanh = mybir.ActivationFunctionType.Tanh
    Exp = mybir.ActivationFunctionType.Exp
    with (
        tc.tile_pool(name="persist", bufs=1) as P,
        tc.tile_pool(name="sums", bufs=3) as sums,
        tc.tile_pool(name="tanhs", bufs=3) as tanhs,
        tc.tile_pool(name="al_ps", bufs=1, space="PSUM") as al_ps,
        tc.tile_pool(name="mm_ps", bufs=2, space="PSUM") as mm_ps,
        tc.tile_pool(name="tr_ps", bufs=2, space="PSUM") as tr_ps,
        tc.tile_pool(name="o_ps", bufs=1, space="PSUM") as o_ps,
    ):
        # ---- persistent SBUF tiles + loads -------------------------------
        # DMA order is priority order: the uh chain (ctxT, wcT) gates the
        # first tanh; epilogue-only tensors (ctx, woutT, bout) are loaded
        # later, overlapped with the main loop.
        def load_wide(name, dram, engine=None):
            # one DMA for a [C*128, F] DRAM tensor -> [128, C*F] SBUF tile,
            # chunk c at free offset c*F (1-2KB contiguous segments)
            rows, F = dram.shape
            C = rows // 128
            t = P.tile([128, C * F], F16, name=name, tag=name)
            eng = engine or nc.sync
            eng.dma_start(
                out=t.rearrange("p (c f) -> p c f", c=C),
                in_=dram.ap().rearrange("(c p) f -> p c f", p=128),
            )
            return t

        ctxT_all = load_wide("ctxT_all", ctxT)
        # wcT and wqT arrive in per-k column pieces, interleaved with the
        # other prologue loads in dependency order: the first tanh quarter
        # only needs the k=0 columns (uh chunk 0 + wqb chunk 0); later
        # chunks land just in time for their prologue phases.
        wcT_all = P.tile([128, NCH * D], F16, name="wcT_all", tag="wcT_all")
        wcT_in3 = wcT.ap().rearrange("(c p) f -> p c f", p=128)
        wcT_out3 = wcT_all.rearrange("p (c f) -> p c f", c=NCH)
        wqT_all = P.tile([128, NCH * D], F16, name="wqT_all", tag="wqT_all")
        wqT_in3 = wqT.ap().rearrange("(c p) f -> p c f", p=128)
        wqT_out3 = wqT_all.rearrange("p (c f) -> p c f", c=NCH)
        nc.scalar.dma_start(out=wcT_out3[:, :, 0:256], in_=wcT_in3[:, :, 0:256])
        nc.scalar.dma_start(out=wqT_out3[:, :, 0:256], in_=wqT_in3[:, :, 0:256])
        inpT_all = load_wide("inpT_all", inpT)
        bq_sb = P.tile([128, NCH], F32, name="bq_sb", tag="bq_sb")
        nc.sync.dma_start(out=bq_sb, in_=bq.ap().rearrange("(k p) -> p k", p=128))
        v_sb = P.tile([128, NCH], F32, name="v_sb", tag="v_sb")
        nc.sync.dma_start(out=v_sb, in_=v.ap().rearrange("(k p) -> p k", p=128))
        nc.scalar.dma_start(out=wcT_out3[:, :, 256:512], in_=wcT_in3[:, :, 256:512])
        nc.scalar.dma_start(out=wqT_out3[:, :, 256:512], in_=wqT_in3[:, :, 256:512])
        ctxT_sb = [ctxT_all[:, ds(S * i, S)] for i in range(NCH)]
        wcT_sb = [wcT_all[:, ds(D * i, D)] for i in range(NCH)]
        wqT_sb = [wqT_all[:, ds(D * i, D)] for i in range(NCH)]
        inpT_sb = [inpT_all[:, ds(TH * i, TH)] for i in range(NCH)]

        # PE warmup first: zero matmuls ramp the PE's continuous-busy clock
        # (max rate after 3us) so the prologue matmuls run at full speed.
        # Emitted before anything DMA-dependent so it starts immediately.
        warm_sb = P.tile([128, S], F16, name="warm_sb", tag="warm_sb")
        nc.vector.memset(warm_sb, 0.0)
        warm_ps = mm_ps.tile([128, S], F32, name="warm_ps", tag="uh_ps")
        for r in range(8):
            nc.tensor.matmul(warm_ps[0:64, :], lhsT=warm_sb[:, 0:64], rhs=warm_sb,
                             start=(r == 0), stop=(r == 7))

        # Z[k]: zeros with v chunk k at column 63; Z[k][:, 63-t:127-t] is a
        # [128, 64] one-hot-column weight whose column t is v chunk k.
        # (zero-fill now; the v column lands after the prologue-critical DVE
        # ops so the v16 copy can't head-block the DVE FIFO)
        Z = []
        for k in range(NCH):
            z = P.tile([128, 2 * TH - 1], F16, name=f"Z{k}", tag=f"Z{k}")
            nc.vector.memset(z, 0.0)
            Z.append(z)

        ident = P.tile([128, 128], F16, name="ident", tag="ident")
        make_identity(nc, ident)
        ones_sb = P.tile([1, TH], F16, name="ones_sb", tag="ones_sb")
        nc.vector.memset(ones_sb, 1.0)

        def load_epilogue_tensors():
            woutT_all = load_wide("woutT_all", woutT, nc.scalar)
            ctx_sb = None
            woutT_sb = [woutT_all[:, ds(D * i, D)] for i in range(2 * NCH)]
            bout_f32 = P.tile([1, D], F32, name="bout_f32", tag="bout_f32")
            nc.sync.dma_start(
                out=bout_f32, in_=bout.ap().rearrange("(o f) -> o f", o=1)
            )
            bout_sb = P.tile([1, D], F16, name="bout_sb", tag="bout_sb")
            nc.vector.tensor_copy(bout_sb, bout_f32)
            return ctx_sb, woutT_sb, bout_sb

        # ---- uh^T[e,s] = Wc @ context^T and wqb^T[e,t] = Wq @ inp^T + bq -
        # Emitted in two phases (chunks 0-1, then 2-3): engine queues are
        # FIFO, so this lets the first tanh (which only needs chunks 0-1 in
        # the k-major layout) start before chunks 2-3 finish.
        uh_sb = [None] * NCH
        wqb_sb = [None] * NCH
        def prologue_phase(ks):
            for k in ks:
                ps = mm_ps.tile([128, S], F32, name="uh_ps", tag="uh_ps")
                for j in range(NCH):
                    nc.tensor.matmul(
                        ps,
                        lhsT=wcT_sb[j][:, ts(k, 128)],
                        rhs=ctxT_sb[j],
                        start=(j == 0),
                        stop=(j == NCH - 1),
                    )
                wps = tr_ps.tile([128, TH], F32, name="wq_ps", tag="wq_ps", bufs=1)
                for j in range(NCH):
                    nc.tensor.matmul(
                        wps,
                        lhsT=wqT_sb[j][:, ts(k, 128)],
                        rhs=inpT_sb[j],
                        start=(j == 0),
                        stop=(j == NCH - 1),
                    )
                # copies/bias-adds on the (prologue-idle) scalar engine to
                # keep the DVE serial chain short
                u = P.tile([128, S], F16, name=f"uh{k}", tag=f"uh{k}")
                nc.vector.tensor_copy(u, ps)
                uh_sb[k] = u
                w = P.tile([128, TH], F32, name=f"wqb{k}", tag=f"wqb{k}")
                nc.vector.tensor_scalar_add(w, wps, bq_sb[:, k : k + 1])
                wqb_sb[k] = w

        prologue_phase([0])

        v16 = P.tile([128, NCH], F16, name="v16", tag="v16")
        nc.vector.tensor_copy(v16, v_sb)
        for k in range(NCH):
            nc.vector.tensor_copy(Z[k][:, TH - 1 : TH], v16[:, k : k + 1])

        # ---- main loop: sum -> tanh -> v-reduction matmuls ---------------
        # Unit u = k*TBLK + tl (k-major) so the first tanh half only needs
        # uh chunks 0..1, letting the stream start before uh chunk 3 lands.
        # align is accumulated in two 32-row PSUM tiles so the epilogue for
        # t 0..31 overlaps the second half of the tanh stream.
        HT = TH // 2  # 32 rows per align half
        al_half = [
            al_ps.tile([HT, S], F32, name=f"al{h}", tag=f"al{h}") for h in range(2)
        ]
        FD = TBLK * NCH * S  # 8192
        NHLF = 2  # ACT instructions per block: keeps PE fed every ~3.6us
        HALF = FD // NHLF
        UPH = TBLK * NCH // NHLF  # (t,k) units per ACT instruction
        ctx_sb = woutT_sb = bout_sb = None

        def epilogue_half(h2, ctx_sb, woutT_sb, bout_sb):
            rows = ds(h2 * HT, HT)
            # softmax over s; no max-subtraction: |align| <= sum|v| (tanh in
            # [-1,1]) is far inside fp32 exp range, and it shortens the
            # critical path.
            p_h = P.tile([HT, S], F32, name=f"p{h2}", tag=f"p{h2}")
            ssum = P.tile([HT, 1], F32, name=f"ssum{h2}", tag=f"ssum{h2}")
            if h2 == 0:
                # mid-stream: skip the accum pass on ACT (shortens the
                # stream insert); the idle DVE does the row-sum instead
                nc.scalar.activation(p_h, al_half[h2], Exp)
                nc.vector.reduce_sum(ssum, p_h, axis=mybir.AxisListType.X)
            else:
                nc.scalar.activation(
                    p_h, al_half[h2], Exp, accum_out=ssum[:, 0:1]
                )
            rcp = P.tile([HT, 1], F32, name=f"rcp{h2}", tag=f"rcp{h2}")
            nc.vector.reciprocal(rcp, ssum)
            a16 = P.tile([HT, S], F16, name=f"a16_{h2}", tag=f"a16_{h2}")
            nc.vector.tensor_scalar_mul(a16, p_h, rcp[:, 0:1])
            nc.vector.tensor_scalar_mul(align_sb[rows, :], p_h, rcp[:, 0:1])
            nc.sync.dma_start(out=align.ap()[rows, :], in_=align_sb[rows, :])

            # alignT[s, t-half] via PE transposes (fp16, one psum tile)
            alT_ps = tr_ps.tile(
                [128, NCH * HT], F16, name="alT_ps", tag="alT_ps", bufs=1
            )
            for i in range(NCH):
                nc.tensor.transpose(
                    alT_ps[:, ts(i, HT)], a16[:, ts(i, 128)], ident[0:HT, 0:HT]
                )
            alT = P.tile([128, NCH * HT], F16, name=f"alT{h2}", tag=f"alT{h2}")
            nc.vector.tensor_copy(alT, alT_ps)

            # attn[t-half, e]: finish the out-projection directly as
            # alignT.T @ M (bias + inp-part already accumulated mid-stream)
            out_ps = out_ps_h[h2]
            for sc in range(NCH):
                nc.tensor.matmul(
                    out_ps,
                    lhsT=alT[:, ts(sc, HT)],
                    rhs=M_sb[sc],
                    start=False,
                    stop=(sc == NCH - 1),
                )
            for eh in range(2):
                ecols = ds(eh * (D // 2), D // 2)
                nc.vector.tensor_copy(attn_sb[rows, ecols], out_ps[:, ecols])
                nc.sync.dma_start(
                    out=attn.ap()[rows, ecols], in_=attn_sb[rows, ecols]
                )

        out_ps_h = {}
        M_sb = [None] * NCH

        def emit_M_chunk(sc, woutT_sb):
            # M[s,e] = sum_f ctx[s,f] * WoutT[f,e]; lhsT = ctxT column slices.
            # Reassociates (align@ctx)@Wout_c = align@M so the tail needs no
            # c-matmul; runs in the PE's mid-stream idle gaps.
            ps = mm_ps.tile([128, S], F32, name="M_ps", tag="uh_ps")
            for j in range(NCH):
                nc.tensor.matmul(
                    ps,
                    lhsT=ctxT_all[:, ds(S * j + 128 * sc, 128)],
                    rhs=woutT_sb[j],
                    start=(j == 0),
                    stop=(j == NCH - 1),
                )
            m = P.tile([128, S], F16, name=f"M{sc}", tag=f"M{sc}")
            nc.vector.tensor_copy(m, ps)
            M_sb[sc] = m

        def out_early(h2, woutT_sb, bout_sb):
            # bias + inp-part of the out-projection depend only on loaded
            # tensors; run them mid-stream so only the c-part is in the tail
            rows = ds(h2 * HT, HT)
            out_ps = o_ps.tile([HT, D], F32, name="out_ps", tag="out_ps", bufs=1)
            nc.tensor.matmul(
                out_ps, lhsT=ones_sb[:, 0:HT], rhs=bout_sb, start=True, stop=False
            )
            for f in range(NCH, 2 * NCH):
                nc.tensor.matmul(
                    out_ps,
                    lhsT=inpT_sb[f - NCH][:, rows],
                    rhs=woutT_sb[f],
                    start=False,
                    stop=False,
                )
            out_ps_h[h2] = out_ps

        align_sb = P.tile([TH, S], F32, name="align_sb", tag="align_sb")
        attn_sb = P.tile([TH, D], F32, name="attn_sb", tag="attn_sb")
        HB = NBLK // 2  # main-loop blocks per align half
        for tb in range(NBLK):
            h2 = tb // HB
            sum_t = sums.tile([128, FD], F16, name="sum_t", tag="sum_t")
            tanh_t = tanhs.tile([128, FD], F16, name="tanh_t", tag="tanh_t")
            if tb == 0:
                # block 0 runs per-chunk quarters with just-in-time prologue
                # phases, so the first tanh only waits for uh chunk 0
                QD = TBLK * S
                for k in range(NCH):
                    for tl in range(TBLK):
                        u = k * TBLK + tl
                        nc.vector.tensor_scalar_add(
                            sum_t[:, ds(u * S, S)], uh_sb[k], wqb_sb[k][:, tl : tl + 1]
                        )
                    if k + 1 < NCH:
                        prologue_phase([k + 1])
                    nc.scalar.activation(
                        tanh_t[:, ds(k * QD, QD)], sum_t[:, ds(k * QD, QD)], Tanh
                    )
                    for tl in range(TBLK):
                        u = k * TBLK + tl
                        nc.tensor.matmul(
                            al_half[0],
                            lhsT=Z[k][:, TH - 1 - tl : TH - 1 - tl + HT],
                            rhs=tanh_t[:, ds(u * S, S)],
                            start=(u == 0),
                            stop=False,
                        )
                # queue the epilogue-only DMAs behind the prologue ones
                ctx_sb, woutT_sb, bout_sb = load_epilogue_tensors()
                continue
            for u in range(TBLK * NCH):
                k, tl = divmod(u, TBLK)
                t = tb * TBLK + tl
                nc.vector.tensor_scalar_add(
                    sum_t[:, ds(u * S, S)], uh_sb[k], wqb_sb[k][:, t : t + 1]
                )
            # the last block runs in quarters so fewer matmuls drain after
            # the final tanh before the B-half softmax can start
            nh = 4 if tb == NBLK - 1 else NHLF
            hfd, uph = FD // nh, TBLK * NCH // nh
            for h in range(nh):
                nc.scalar.activation(
                    tanh_t[:, ds(h * hfd, hfd)], sum_t[:, ds(h * hfd, hfd)], Tanh
                )
                for u in range(h * uph, (h + 1) * uph):
                    k, tl = divmod(u, TBLK)
                    t_loc = (tb % HB) * TBLK + tl
                    nc.tensor.matmul(
                        al_half[h2],
                        lhsT=Z[k][:, TH - 1 - t_loc : TH - 1 - t_loc + HT],
                        rhs=tanh_t[:, ds(u * S, S)],
                        start=(tb % HB == 0 and u == 0),
                        stop=(tb % HB == HB - 1 and u == TBLK * NCH - 1),
                    )
            if 2 <= tb <= 5:
                emit_M_chunk(tb - 2, woutT_sb)
            if tb == HB - 4:
                out_early(0, woutT_sb, bout_sb)
            if tb == NBLK - 4:
                out_early(1, woutT_sb, bout_sb)
            if tb % HB == HB - 1:
                epilogue_half(h2, ctx_sb, woutT_sb, bout_sb)


def get_nc():
    if "nc" not in _NC_CACHE:
        _NC_CACHE["nc"] = _build_nc()
    return _NC_CACHE["nc"]


def make_in_maps(inp, context, Wq, bq, Wc, v, Wout, bout):
    inp = np.asarray(inp, np.float32)
    context = np.asarray(context, np.float32)
    Wq = np.asarray(Wq, np.float32)
    bq = np.asarray(bq, np.float32)
    Wc = np.asarray(Wc, np.float32)
    v = np.asarray(v, np.float32)
    Wout = np.asarray(Wout, np.float32)
    bout = np.asarray(bout, np.float32)

    wqT = np.ascontiguousarray(Wq.T).astype(np.float16)
    wcT = np.ascontiguousarray(Wc.T).astype(np.float16)
    woutT = np.ascontiguousarray(Wout.T).astype(np.float16)
    in_maps = []
    for c in range(N_CORES):
        b, th = divmod(c, 2)
        in_maps.append(
            {
                "inpT": np.ascontiguousarray(
                    inp[b, th * TH : (th + 1) * TH].T
                ).astype(np.float16),
                "ctxT": np.ascontiguousarray(context[b].T).astype(np.float16),
                "wqT": wqT,
                "wcT": wcT,
                "woutT": woutT,
                "bq": bq,
                "v": v,
                "bout": bout,
            }
        )
    return in_maps


def run_on_device(in_maps, **kwargs):
    nc = get_nc()
    return run_bass_kernel_spmd(nc, in_maps, core_ids=list(range(N_CORES)), **kwargs)


def kernel(inp, context, Wq, bq, Wc, v, Wout, bout):
    in_maps = make_in_maps(inp, context, Wq, bq, Wc, v, Wout, bout)
    res = run_on_device(in_maps)
    attn = np.empty((B, T, D), np.float32)
    align = np.empty((B, T, S), np.float32)
    for c in range(N_CORES):
        b, th = divmod(c, 2)
        attn[b, th * TH : (th + 1) * TH] = res.results[c]["attn"]
        align[b, th * TH : (th + 1) * TH] = res.results[c]["align"]
    return attn, align



# revision 9
# speedup vs baseline: 2.6175x; 2.6175x over previous
"""Bahdanau (additive) attention kernel for Trainium2, 8 NeuronCores.

Problem shapes: inp (B=4, T=128, D=512), context (B=4, S=512, D=512).
  wq   = inp @ Wq.T + bq                      (B,T,D)
  uh   = context @ Wc.T                       (B,S,D)
  align= einsum('btsd,d->bts', tanh(wq[:,:,None,:]+uh[:,None,:,:]), v)
  a    = softmax(align, -1)                   (B,T,S)
  c    = einsum('bts,bsd->btd', a, context)
  attn = concat([c, inp], -1) @ Wout.T + bout (B,T,D)
Returns (attn, a).

Algorithm: the O(T*S*D) tanh stream is replaced by a separable sinusoid
expansion.  tanh(x) ~ sum_j c_j sin(n_j w x) with odd harmonics
n_j = 2j-1 (the Neumann-reflected periodic extension of tanh on [-L,L]
has only odd terms), so with a = wq, b = uh:

  align[t,s] = sum_d v_d tanh(a_td + b_sd)
            ~= sum_{d,j} [c_j sin(n_j w a_td)] [v_d cos(n_j w b_sd)]
             + [c_j cos(n_j w a_td)] [v_d sin(n_j w b_sd)]

which is ONE PE matmul with contraction (d, j, phase) = 2*J*D, plus
O((T+S)*D*J) trig features.  Features are generated by ACT seeds
(sin at w/2 and w; cos/E2 via Square, so every ACT arg stays in
[-pi,pi]) and fp16 Chebyshev ladders sin_{n+2} = E2*sin_n - sin_{n-2}
on DVE/Pool.  v rides the (linear) B-side ladder seeds; c_j is applied
to A-side features.

Sharding: 8 cores = (batch b, source-half sh); each core computes its
[T=128, SH=256] block of unnormalized p = exp(align), the partial row
sums sig, the partial output V = p16 @ M (M = ctx_half @ WoutC), and
I = inp@WoutI + bout.  The host finishes the softmax / output reduction
at gather time: attn = (V0+V1)/(sig0+sig1) + I, align = p/(sig0+sig1).
"""

import numpy as np

import concourse.bacc as bacc
import concourse.tile as tile
from concourse import mybir
from concourse.bass import ds, ts
from concourse.bass_utils import run_bass_kernel_spmd
from concourse.masks import make_identity

F32 = mybir.dt.float32
F16 = mybir.dt.float16
ALU = mybir.AluOpType

B, T, S, D = 4, 128, 512, 512
SH = S // 2  # source positions per core
N_CORES = 8
NCH = D // 128  # partition chunks of the model dim

J = 7
L = 6.0
OM = float(np.pi / (2 * L))
CS = [1.23838309, 0.33377506, 0.13636649, 0.05874373,
      0.02549577, 0.0112301, 0.00513364][:J]

_NC_CACHE = {}


def _build_nc():
    nc = bacc.Bacc("TRN2", target_bir_lowering=False, debug=False, num_devices=N_CORES)

    ctxT = nc.dram_tensor("ctxT", [D, SH], F16, kind="ExternalInput")
    wcT = nc.dram_tensor("wcT", [D, D], F16, kind="ExternalInput")
    wqT = nc.dram_tensor("wqT", [D, D], F16, kind="ExternalInput")
    inpT = nc.dram_tensor("inpT", [D, T], F16, kind="ExternalInput")
    woutT = nc.dram_tensor("woutT", [2 * D, D], F16, kind="ExternalInput")
    bq = nc.dram_tensor("bq", [D], F32, kind="ExternalInput")
    v = nc.dram_tensor("v", [D], F32, kind="ExternalInput")
    bout = nc.dram_tensor("bout", [D], F32, kind="ExternalInput")
    p_out = nc.dram_tensor("p_out", [T, SH], F32, kind="ExternalOutput")
    sig = nc.dram_tensor("sig", [T, 1], F32, kind="ExternalOutput")
    V_out = nc.dram_tensor("V_out", [T, D], F32, kind="ExternalOutput")
    I_out = nc.dram_tensor("I_out", [T, D], F32, kind="ExternalOutput")

    with tile.TileContext(nc) as tc:
        _emit(nc, tc, ctxT, wcT, wqT, inpT, woutT, bq, v, bout,
              p_out, sig, V_out, I_out)
    nc.compile()
    return nc


def _emit(nc, tc, ctxT, wcT, wqT, inpT, woutT, bq, v, bout,
          p_out, sig, V_out, I_out):
    Sin = mybir.ActivationFunctionType.Sin
    Sq = mybir.ActivationFunctionType.Square
    Exp = mybir.ActivationFunctionType.Exp
    with (
        tc.tile_pool(name="persist", bufs=1) as P,
        tc.tile_pool(name="uh_ps", bufs=1, space="PSUM") as uh_pool,
        tc.tile_pool(name="wq_ps", bufs=1, space="PSUM") as wq_pool,
        tc.tile_pool(name="al_ps", bufs=1, space="PSUM") as al_pool,
        tc.tile_pool(name="ep_ps", bufs=2, space="PSUM") as ep_pool,
    ):
        # ---- persistent SBUF tiles + loads -------------------------------
        def load_wide(name, dram, engine=None):
            rows, F = dram.shape
            C = rows // 128
            t = P.tile([128, C * F], F16, name=name, tag=name)
            eng = engine or nc.sync
            eng.dma_start(
                out=t.rearrange("p (c f) -> p c f", c=C),
                in_=dram.ap().rearrange("(c p) f -> p c f", p=128),
            )
            return t

        ctxT_all = load_wide("ctxT_all", ctxT)          # [128, 4*SH]
        wcT_all = load_wide("wcT_all", wcT)             # [128, 4*D]
        wqT_all = load_wide("wqT_all", wqT, nc.scalar)  # [128, 4*D]
        inpT_all = load_wide("inpT_all", inpT, nc.scalar)  # [128, 4*T]
        v_sb = P.tile([128, NCH], F32, name="v_sb", tag="v_sb")
        nc.sync.dma_start(out=v_sb, in_=v.ap().rearrange("(k p) -> p k", p=128))
        bq_sb = P.tile([128, NCH], F32, name="bq_sb", tag="bq_sb")
        nc.sync.dma_start(out=bq_sb, in_=bq.ap().rearrange("(k p) -> p k", p=128))

        ctxT_sb = [ctxT_all[:, ds(SH * i, SH)] for i in range(NCH)]
        wcT_sb = [wcT_all[:, ds(D * i, D)] for i in range(NCH)]
        wqT_sb = [wqT_all[:, ds(D * i, D)] for i in range(NCH)]
        inpT_sb = [inpT_all[:, ds(T * i, T)] for i in range(NCH)]

        # PE warmup: zero matmuls ramp the HAM clock before real work.
        warm_sb = P.tile([128, SH], F16, name="warm_sb", tag="warm_sb")
        nc.vector.memset(warm_sb, 0.0)
        warm_ps = ep_pool.tile([128, SH], F32, name="warm_ps", tag="ep")
        for r in range(8):
            nc.tensor.matmul(warm_ps[0:64, :], lhsT=warm_sb[:, 0:64], rhs=warm_sb,
                             start=(r == 0), stop=(r == 7))

        ident = P.tile([128, 128], F16, name="ident", tag="ident")
        make_identity(nc, ident)
        ones_sb = P.tile([1, T], F16, name="ones_sb", tag="ones_sb")
        nc.vector.memset(ones_sb, 1.0)

        # ---- uh^T = Wc @ ctx^T (4 chunks), wq^T = Wq @ inp^T -------------
        uh_wide = uh_pool.tile([128, NCH * SH], F32, name="uh_wide", tag="uh")
        uh_ps = [uh_wide[:, ts(k, SH)] for k in range(NCH)]
        for k in range(NCH):
            for j in range(NCH):
                nc.tensor.matmul(uh_ps[k], lhsT=wcT_sb[j][:, ts(k, 128)],
                                 rhs=ctxT_sb[j], start=(j == 0), stop=(j == NCH - 1))
        wq_wide = wq_pool.tile([128, NCH * T], F32, name="wq_wide", tag="wq")
        wq_ps = [wq_wide[:, ts(k, T)] for k in range(NCH)]
        for k in range(NCH):
            for j in range(NCH):
                nc.tensor.matmul(wq_ps[k], lhsT=wqT_sb[j][:, ts(k, 128)],
                                 rhs=inpT_sb[j], start=(j == 0), stop=(j == NCH - 1))

        # ---- ACT seeds (all args within [-pi, pi]) -----------------------
        # B side: sh_B = sin(w/2 uh), s1 = sin(w uh) read straight from PSUM
        shB = P.tile([128, NCH * SH], F16, name="shB", tag="shB")
        s1Br = P.tile([128, NCH * SH], F16, name="s1Br", tag="s1Br")
        for k in range(NCH):
            nc.scalar.activation(shB[:, ts(k, SH)], uh_ps[k], Sin, scale=OM / 2)
        for k in range(NCH):
            nc.scalar.activation(s1Br[:, ts(k, SH)], uh_ps[k], Sin, scale=OM)
        # cos1 = 1 - 2 sin(w/2)^2 ; E2 = 2 cos(2w uh) = 2 - 4 sin(w uh)^2
        qB = P.tile([128, NCH * SH], F16, name="qB", tag="qB")
        nc.scalar.activation(qB, shB, Sq)
        c1Br = P.tile([128, NCH * SH], F16, name="c1Br", tag="c1Br")
        nc.vector.tensor_scalar(c1Br, qB, -2.0, 1.0, ALU.mult, ALU.add)
        qB2 = P.tile([128, NCH * SH], F16, name="qB2", tag="qB2")
        nc.scalar.activation(qB2, s1Br, Sq)
        E2B = P.tile([128, NCH * SH], F16, name="E2B", tag="E2B")
        nc.vector.tensor_scalar(E2B, qB2, -4.0, 2.0, ALU.mult, ALU.add)

        # A side: wqb = wq + bq (f32), then seeds
        wqb = P.tile([128, NCH * T], F32, name="wqb", tag="wqb")
        for k in range(NCH):
            nc.vector.tensor_scalar_add(wqb[:, ts(k, T)], wq_ps[k],
                                        bq_sb[:, k:k + 1])
        shA = P.tile([128, NCH * T], F16, name="shA", tag="shA")
        nc.scalar.activation(shA, wqb, Sin, scale=OM / 2)
        s1Ar = P.tile([128, NCH * T], F16, name="s1Ar", tag="s1Ar")
        nc.scalar.activation(s1Ar, wqb, Sin, scale=OM)
        qA = P.tile([128, NCH * T], F16, name="qA", tag="qA")
        nc.scalar.activation(qA, shA, Sq)
        c1Ar = P.tile([128, NCH * T], F16, name="c1Ar", tag="c1Ar")
        nc.vector.tensor_scalar(c1Ar, qA, -2.0, 1.0, ALU.mult, ALU.add)
        qA2 = P.tile([128, NCH * T], F16, name="qA2", tag="qA2")
        nc.scalar.activation(qA2, s1Ar, Sq)
        E2A = P.tile([128, NCH * T], F16, name="E2A", tag="E2A")
        nc.vector.tensor_scalar(E2A, qA2, -4.0, 2.0, ALU.mult, ALU.add)

        # ---- feature tiles ----------------------------------------------
        # B features carry v (per-partition, folded into the linear ladder
        # seeds); A features carry c_j (uniform scalar per feature).
        Bs = [P.tile([128, NCH * SH], F16, name=f"Bs{j}", tag=f"Bs{j}")
              for j in range(J)]
        Bc = [P.tile([128, NCH * SH], F16, name=f"Bc{j}", tag=f"Bc{j}")
              for j in range(J)]
        As = [P.tile([128, NCH * T], F16, name=f"As{j}", tag=f"As{j}")
              for j in range(J)]
        Ac = [P.tile([128, NCH * T], F16, name=f"Ac{j}", tag=f"Ac{j}")
              for j in range(J)]
        # raw (unscaled) A ladder state; slot 0 holds the raw seeds
        Asr = [s1Ar] + [P.tile([128, NCH * T], F16, name=f"Asr{j}", tag=f"Asr{j}")
                        for j in range(1, J)]
        Acr = [c1Ar] + [P.tile([128, NCH * T], F16, name=f"Acr{j}", tag=f"Acr{j}")
                        for j in range(1, J)]
        tmp_pool = [P.tile([128, NCH * SH], F16, name=f"lt{i}", tag=f"lt{i}")
                    for i in range(4)]

        # v-scaled B seeds
        for k in range(NCH):
            nc.vector.tensor_scalar_mul(Bs[0][:, ts(k, SH)], s1Br[:, ts(k, SH)],
                                        v_sb[:, k:k + 1])
        for k in range(NCH):
            nc.vector.tensor_scalar_mul(Bc[0][:, ts(k, SH)], c1Br[:, ts(k, SH)],
                                        v_sb[:, k:k + 1])
        # c-scaled A seeds (j=0 features)
        nc.vector.tensor_scalar_mul(As[0], s1Ar, CS[0])
        nc.vector.tensor_scalar_mul(Ac[0], c1Ar, CS[0])

        # ---- epilogue operand prep (runs on PE idle gaps) ----------------
        woutT_all = load_wide("woutT_all", woutT, nc.scalar)
        woutT_sb = [woutT_all[:, ds(D * i, D)] for i in range(2 * NCH)]
        bout_f32 = P.tile([1, D], F32, name="bout_f32", tag="bout_f32")
        nc.scalar.dma_start(out=bout_f32, in_=bout.ap().rearrange("(o f) -> o f", o=1))
        bout_sb = P.tile([1, D], F16, name="bout_sb", tag="bout_sb")
        nc.gpsimd.tensor_copy(bout_sb, bout_f32)

        align_ps = al_pool.tile([T, SH], F32, name="align", tag="align")

        def align_mm(j, start, stop):
            # align += As_j^T Bc_j + Ac_j^T Bs_j over the 4 d-chunks
            for k in range(NCH):
                nc.tensor.matmul(align_ps, lhsT=As[j][:, ts(k, T)],
                                 rhs=Bc[j][:, ts(k, SH)],
                                 start=start and k == 0, stop=False)
            for k in range(NCH):
                nc.tensor.matmul(align_ps, lhsT=Ac[j][:, ts(k, T)],
                                 rhs=Bs[j][:, ts(k, SH)],
                                 start=False, stop=stop and k == NCH - 1)

        align_mm(0, True, False)

        # ---- ladders + align accumulation -------------------------------
        # step j (0-indexed features; harmonic n = 2j+1):
        #   X[j] = E2*X[j-1] - X[j-2],  with X[-1] = -X[0] for sin,
        #   +X[0] for cos  (handled by add/sub choice at j==1).
        def ladder_step(j, X, is_sin, E2, eng, tmp):
            op2 = ALU.add if (j == 1 and is_sin) else ALU.subtract
            prev2 = X[0] if j == 1 else X[j - 2]
            eng.tensor_tensor(tmp, E2, X[j - 1], ALU.mult)
            eng.tensor_tensor(X[j], tmp, prev2, op2)

        for j in range(1, J):
            # B chains on DVE (big tiles), A chains split DVE/Pool
            ladder_step(j, Bs, True, E2B, nc.vector, tmp_pool[0])
            ladder_step(j, Bc, False, E2B, nc.vector, tmp_pool[1])
            ladder_step(j, Asr, True, E2A, nc.vector, tmp_pool[2][:, 0:NCH * T])
            ladder_step(j, Acr, False, E2A, nc.gpsimd, tmp_pool[3][:, 0:NCH * T])
            # apply c_j to A features
            nc.vector.tensor_scalar_mul(As[j], Asr[j], CS[j])
            nc.gpsimd.tensor_scalar_mul(Ac[j], Acr[j], CS[j])
            align_mm(j, False, j == J - 1)
            if j == 1:
                M_sb = emit_M(nc, ctxT_all, woutT_sb, P, ep_pool)
            if j == 2:
                emit_I(nc, inpT_sb, woutT_sb, bout_sb, ones_sb, P, ep_pool,
                       I_out)

        # ---- epilogue: p = exp(align), sig, V = p16 @ M ------------------
        p32 = P.tile([T, SH], F32, name="p32", tag="p32")
        sig_sb = P.tile([T, 1], F32, name="sig_sb", tag="sig_sb")
        nc.scalar.activation(p32, align_ps, Exp, accum_out=sig_sb[:, 0:1])
        nc.sync.dma_start(out=sig.ap(), in_=sig_sb)
        nc.sync.dma_start(out=p_out.ap(), in_=p32)
        p16 = P.tile([T, SH], F16, name="p16", tag="p16")
        nc.vector.tensor_copy(p16, p32)

        # pT via PE transposes
        pT_ps = ep_pool.tile([128, 2 * T], F16, name="pT_ps", tag="ep")
        for i in range(2):
            nc.tensor.transpose(pT_ps[:, ts(i, T)], p16[:, ts(i, 128)],
                                ident[0:T, 0:T])
        pT_sb = P.tile([128, 2 * T], F16, name="pT_sb", tag="pT_sb")
        nc.vector.tensor_copy(pT_sb, pT_ps)

        V_ps = ep_pool.tile([T, D], F32, name="V_ps", tag="ep")
        for i in range(2):
            nc.tensor.matmul(V_ps, lhsT=pT_sb[:, ts(i, T)],
                             rhs=M_sb[:, ts(i, D)],
                             start=(i == 0), stop=(i == 1))
        V_sb = P.tile([T, D], F32, name="V_sb", tag="V_sb")
        nc.vector.tensor_copy(V_sb, V_ps)
        nc.sync.dma_start(out=V_out.ap(), in_=V_sb)


def emit_M(nc, ctxT_all, woutT_sb, P, ep_pool):
    # M[s, e] = sum_f ctx[s, f] Wout_c[e, f]; s-half rows in 2 chunks.
    M_sb = P.tile([128, 2 * D], F16, name="M_sb", tag="M_sb")
    for sc in range(2):
        ps = ep_pool.tile([128, D], F32, name=f"M{sc}", tag="ep")
        for j in range(NCH):
            nc.tensor.matmul(ps, lhsT=ctxT_all[:, ds(SH * j + 128 * sc, 128)],
                             rhs=woutT_sb[j], start=(j == 0), stop=(j == NCH - 1))
        nc.scalar.copy(M_sb[:, ts(sc, D)], ps)
    return M_sb


def emit_I(nc, inpT_sb, woutT_sb, bout_sb, ones_sb, P, ep_pool, I_out):
    # I = inp @ WoutI + bout
    ps = ep_pool.tile([T, D], F32, name="I_ps", tag="ep")
    nc.tensor.matmul(ps, lhsT=ones_sb[:, 0:T], rhs=bout_sb, start=True, stop=False)
    for f in range(NCH):
        nc.tensor.matmul(ps, lhsT=inpT_sb[f], rhs=woutT_sb[NCH + f],
                         start=False, stop=(f == NCH - 1))
    I_sb = P.tile([T, D], F32, name="I_sb", tag="I_sb")
    nc.scalar.copy(I_sb, ps)
    nc.scalar.dma_start(out=I_out.ap(), in_=I_sb)


def get_nc():
    if "nc" not in _NC_CACHE:
        _NC_CACHE["nc"] = _build_nc()
    return _NC_CACHE["nc"]


def make_in_maps(inp, context, Wq, bq, Wc, v, Wout, bout):
    inp = np.asarray(inp, np.float32)
    context = np.asarray(context, np.float32)
    wqT = np.ascontiguousarray(np.asarray(Wq, np.float32).T).astype(np.float16)
    wcT = np.ascontiguousarray(np.asarray(Wc, np.float32).T).astype(np.float16)
    woutT = np.ascontiguousarray(np.asarray(Wout, np.float32).T).astype(np.float16)
    bq = np.asarray(bq, np.float32)
    v = np.asarray(v, np.float32)
    bout = np.asarray(bout, np.float32)
    in_maps = []
    for c in range(N_CORES):
        b, sh = divmod(c, 2)
        in_maps.append({
            "ctxT": np.ascontiguousarray(
                context[b].T[:, sh * SH:(sh + 1) * SH]).astype(np.float16),
            "wcT": wcT,
            "wqT": wqT,
            "inpT": np.ascontiguousarray(inp[b].T).astype(np.float16),
            "woutT": woutT,
            "bq": bq,
            "v": v,
            "bout": bout,
        })
    return in_maps


def run_on_device(in_maps, **kwargs):
    nc = get_nc()
    return run_bass_kernel_spmd(nc, in_maps, core_ids=list(range(N_CORES)), **kwargs)


def kernel(inp, context, Wq, bq, Wc, v, Wout, bout):
    in_maps = make_in_maps(inp, context, Wq, bq, Wc, v, Wout, bout)
    res = run_on_device(in_maps)
    attn = np.empty((B, T, D), np.float32)
    align = np.empty((B, T, S), np.float32)
    for b in range(B):
        r0 = res.results[2 * b]
        r1 = res.results[2 * b + 1]
        stot = r0["sig"] + r1["sig"]  # (T,1)
        attn[b] = (r0["V_out"] + r1["V_out"]) / stot + r0["I_out"]
        align[b, :, :SH] = r0["p_out"] / stot
        align[b, :, SH:] = r1["p_out"] / stot
    return attn, align
